# revision 16
# baseline (speedup 1.0000x reference)
"""Trainium2 Bass kernel for nn_Deformable_33397665693799.

Strategy (8 cores, B=4): 2 cores per batch; each core computes the full
per-batch shared pipeline (LN1 -> Q -> depthwise-conv offsets -> deformed
grid-sample gather -> V/KH/VH) and half of the 4096 attention queries
(attention + output projections + MLP tail).  The query halves are selected
without any per-core program differences by feeding each core a
channel-ROTATED copy of x: the faithful torch-style reshape scramble maps
token t = 8c+s to LN-channel c, so rotating x's channels by 256h makes each
core's "first 2048 tokens" equal true tokens [2048h, 2048h+2048).  The
depthwise-conv weights / 1x1-offset-conv weights are rotated identically on
the host, and grid-sample keys are an (order-irrelevant) permutation, so the
single SPMD program is exact for both halves.

All cores run one identical Bass/Tile program; only input data differs.
"""
import sys
import os

sys.path.insert(0, "/opt/trn_rl_repo")

import numpy as np
import ml_dtypes

import concourse.bass as bass
import concourse.mybir as mybir
import concourse.tile as tile
from concourse import bacc
from concourse.masks import make_identity

FP32 = mybir.dt.float32
BF16 = mybir.dt.bfloat16
I32 = mybir.dt.int32
ALU = mybir.AluOpType
ACTF = mybir.ActivationFunctionType

P = 128
C = 512            # channels
T = 4096           # tokens per batch
TH = 2048          # tokens per core (query half)
KEYS = 1024        # attention keys
NH = 8             # heads
PADW = 70          # padded conv row width  (x in [-3, 67))
PADH = 70          # padded conv rows       (y in [-3, 67))
CONVF = PADH * PADW


def build_program():
    nc = bacc.Bacc("TRN2", target_bir_lowering=False, debug=False)

    # ---------------- DRAM I/O ----------------
    d_x = nc.dram_tensor("x_rot", [T, C], FP32, kind="ExternalInput")
    d_xgat = nc.dram_tensor("x_gat", [T + 1, C], BF16, kind="ExternalInput")
    d_xres = nc.dram_tensor("x_res", [TH, C], FP32, kind="ExternalInput")
    d_w = {}
    for name in ("wq", "wv", "mq", "mk", "mv", "mo", "mlp"):
        d_w[name] = nc.dram_tensor(name, [C, C], BF16, kind="ExternalInput")
    d_dwdiag = nc.dram_tensor("dwdiag", [P, 49 * 4 * P], BF16, kind="ExternalInput")
    d_pw = nc.dram_tensor("pw", [C, 2], BF16, kind="ExternalInput")
    d_refsA = nc.dram_tensor("refsA", [P, 8], FP32, kind="ExternalInput")
    d_refsB = nc.dram_tensor("refsB", [P, 8], FP32, kind="ExternalInput")
    d_out = nc.dram_tensor("out", [TH, C], FP32, kind="ExternalOutput")

    with tile.TileContext(nc) as tc:
        drs = tc.alloc_tile_pool(name="drs", bufs=1, space="DRAM")
        pers = tc.alloc_tile_pool(name="persist", bufs=1)

        q_scr4 = [drs.tile([T // 4, C], BF16, name=f"q_scr{i}")
                  for i in range(4)]
        scr_off = drs.tile([2048], FP32, name="scr_off")

        # ---- persistent (whole-kernel) SBUF ----
        w_sb = {}
        for name in ("wq", "wv", "mq", "mk", "mv", "mo", "mlp"):
            w_sb[name] = pers.tile([P, 4 * C], BF16, name=f"w_{name}",
                                   tag=f"w_{name}")
            for a in range(4):
                nc.sync.dma_start(w_sb[name][:, a * C:(a + 1) * C],
                                  d_w[name][a * P:(a + 1) * P, :])
        pw_sb = pers.tile([P, 8], BF16, name="pw_sb", tag="pw_sb")
        for a in range(4):
            nc.sync.dma_start(pw_sb[:, a * 2:(a + 1) * 2],
                              d_pw[a * P:(a + 1) * P, :])
        refsA = pers.tile([P, 8], FP32, name="refsA", tag="refsA")
        refsB = pers.tile([P, 8], FP32, name="refsB", tag="refsB")
        nc.sync.dma_start(refsA[:], d_refsA[:])
        nc.sync.dma_start(refsB[:], d_refsB[:])
        ident = pers.tile([P, P], BF16, name="ident", tag="ident")
        make_identity(nc, ident[:])

        kht = pers.tile([P, 4 * KEYS], BF16, name="kht", tag="kht")
        vt = pers.tile([P, 4 * KEYS], BF16, name="vt", tag="vt")
        vh65 = pers.tile([P, 8 * 520], BF16, name="vh65", tag="vh65")
        interp = pers.tile([P, 8 * C], BF16, name="interp", tag="interp")
        qht = pers.tile([P, 4 * TH], BF16, name="qht", tag="qht")

        # ---- scoped big buffers ----
        pool_ln1 = tc.alloc_tile_pool(name="p_ln1", bufs=1)
        ln1 = pool_ln1.tile([P, 32 * C], BF16, name="ln1", tag="ln1")
        pool_qt = tc.alloc_tile_pool(name="p_qt", bufs=1, side="right")
        qt = pool_qt.tile([P, 4 * T], BF16, name="qt", tag="qt")

        # ---------------- Stage 1: LN1 ----------------
        with tc.tile_pool(name="s1", bufs=3) as s1, \
             tc.tile_pool(name="s1s", bufs=4) as s1s:
            for pi in range(32):
                xt = s1.tile([P, C], FP32, name="xt", tag="xt")
                nc.sync.dma_start(xt[:], d_x[pi * P:(pi + 1) * P, :])
                stats = s1s.tile([P, 6], FP32, name="stats", tag="stats")
                aggr = s1s.tile([P, 2], FP32, name="aggr", tag="aggr")
                rstd = s1s.tile([P, 1], FP32, name="rstd", tag="rstd")
                eps = s1s.tile([P, 1], FP32, name="eps", tag="eps")
                nc.vector.bn_stats(stats[:], xt[:])
                nc.vector.bn_aggr(aggr[:], stats[:])
                nc.any.memset(eps[:], 1e-5)
                nc.scalar.activation(rstd[:], aggr[:, 1:2], ACTF.Sqrt,
                                     bias=eps[:, 0:1], scale=1.0)
                nc.vector.reciprocal(rstd[:], rstd[:])
                nc.vector.tensor_scalar(
                    ln1[:, pi * C:(pi + 1) * C], xt[:],
                    aggr[:, 0:1], rstd[:, 0:1], ALU.subtract, ALU.mult)

        # ---------------- Stage 2: Q^T ----------------
        qtv = qt[:].rearrange("p (cp c s8) -> p cp c s8", cp=4, s8=8)
        with tc.tile_pool(name="s2p", bufs=8, space="PSUM") as s2p:
            for cp in range(4):
                psums = [s2p.tile([P, C], FP32, name="qpsum", tag="qpsum")
                         for _ in range(8)]
                for a in range(4):
                    lhsT = w_sb["wq"][:, a * C + cp * P: a * C + (cp + 1) * P]
                    for s in range(8):
                        nc.tensor.matmul(
                            psums[s][:], lhsT,
                            ln1[:, (4 * s + a) * C:(4 * s + a + 1) * C],
                            start=(a == 0), stop=(a == 3))
                for s in range(8):
                    nc.vector.tensor_copy(qtv[:, cp, :, s], psums[s][:])
        pool_ln1.release()

        # ---------------- Stage 3: Q^T -> q_scr -> padded conv input --------
        pool_conv = tc.alloc_tile_pool(name="p_conv", bufs=1, side="right")
        conv_in = pool_conv.tile([P, 4 * CONVF], BF16, name="conv_in",
                                 tag="conv_in")
        gelu_sb = pool_conv.tile([P, 4 * 1024], BF16, name="gelu_sb",
                                 tag="gelu_sb")
        nc.gpsimd.memset(conv_in[:], 0.0)
        with tc.tile_pool(name="s3", bufs=4) as s3, \
             tc.tile_pool(name="s3p", bufs=4, space="PSUM") as s3p:
            for tt in range(32):
                pst = s3p.tile([P, C], BF16, name="pst", tag="pst")
                for cp in range(4):
                    nc.tensor.transpose(
                        pst[:, cp * P:(cp + 1) * P],
                        qt[:, cp * T + tt * P: cp * T + (tt + 1) * P],
                        ident[:])
                qtmp = s3.tile([P, C], BF16, name="qtmp", tag="qtmp")
                nc.scalar.copy(qtmp[:], pst[:])
                T4, rr = tt // 8, tt % 8
                nc.sync.dma_start(q_scr4[T4][rr * P:(rr + 1) * P, :], qtmp[:])
                if rr == 7:
                    civ = conv_in[:, T4 * CONVF:(T4 + 1) * CONVF] \
                        .rearrange("p (y x) -> p y x", y=PADH)
                    qs_view = q_scr4[T4][:] \
                        .rearrange("(c s) j -> c (s j)", s=8) \
                        .rearrange("c (y x) -> c y x", y=64)
                    nc.sync.dma_start(civ[:, 3:67, 3:67], qs_view[:])

        # ---------------- Stage 4: depthwise conv 7x7 stride 2 -------------
        with tc.tile_pool(name="s4d", bufs=6) as s4d, \
             tc.tile_pool(name="s4p", bufs=4, space="PSUM") as s4p:
            cpsum = [s4p.tile([P, 1024], FP32, name="cpsum", tag="cpsum")
                     for _ in range(4)]
            dg8 = None
            for tap in range(49):
                ky, kx = tap // 7, tap % 7
                for T4 in range(4):
                    k8 = (tap * 4 + T4) % 8
                    if k8 == 0:
                        dg8 = s4d.tile([P, 8 * P], BF16, name="dg8", tag="dg8")
                        col0 = (tap * 4 + T4) * P
                        ncol = min(8 * P, 49 * 4 * P - col0)
                        nc.sync.dma_start(dg8[:, 0:ncol],
                                          d_dwdiag[:, col0:col0 + ncol])
                    dg = dg8[:, k8 * P:(k8 + 1) * P]
                    civ = conv_in[:, T4 * CONVF:(T4 + 1) * CONVF] \
                        .rearrange("p (y x) -> p y x", y=PADH)
                    rv = civ[:, ky:ky + 64:2, kx:kx + 64:2]
                    nc.tensor.matmul(cpsum[T4][:, 0:512], dg,
                                     rv[:, 0:16, :],
                                     start=(tap == 0), stop=(tap == 48))
                    nc.tensor.matmul(cpsum[T4][:, 512:1024], dg,
                                     rv[:, 16:32, :],
                                     start=(tap == 0), stop=(tap == 48))
            for T4 in range(4):
                nc.scalar.activation(gelu_sb[:, T4 * 1024:(T4 + 1) * 1024],
                                     cpsum[T4][:], ACTF.Gelu)

        # ---------------- Stage 5: offsets + tanh ---------------------------
        with tc.tile_pool(name="s5p", bufs=1, space="PSUM") as s5p, \
             tc.tile_pool(name="s5", bufs=1) as s5:
            opsum = s5p.tile([2, 1024], FP32, name="opsum", tag="opsum")
            for T4 in range(4):
                for half in range(2):
                    nc.tensor.matmul(
                        opsum[:, half * 512:(half + 1) * 512],
                        pw_sb[:, T4 * 2:(T4 + 1) * 2],
                        gelu_sb[:, T4 * 1024 + half * 512:
                                T4 * 1024 + (half + 1) * 512],
                        start=(T4 == 0), stop=(T4 == 3))
            off_t = s5.tile([2, 1024], FP32, name="off_t", tag="off_t")
            nc.scalar.activation(off_t[:], opsum[:], ACTF.Tanh)
            nc.sync.dma_start(scr_off[0:1024], off_t[0:1, :])
            nc.sync.dma_start(scr_off[1024:2048], off_t[1:2, :])
        pool_conv.release()

        # ---------------- Stage 5b: QH^T (overlaps gather) ------------------
        with tc.tile_pool(name="s5q", bufs=4, space="PSUM") as s5q:
            for fp in range(4):
                for ch in range(4):
                    ps = s5q.tile([P, 512], FP32, name="qhps", tag="qhps")
                    for a in range(4):
                        nc.tensor.matmul(
                            ps[:],
                            w_sb["mq"][:, a * C + fp * P: a * C + (fp + 1) * P],
                            qt[:, a * T + ch * 512:a * T + (ch + 1) * 512],
                            start=(a == 0), stop=(a == 3))
                    nc.vector.tensor_copy(
                        qht[:, fp * TH + ch * 512: fp * TH + (ch + 1) * 512],
                        ps[:])

        # ---------------- Stage 6+7: pixel math, gather, bilinear -----------
        with tc.tile_pool(name="s6", bufs=1) as s6:
            tA = s6.tile([P, 8], FP32, name="tA", tag="tA")
            tB = s6.tile([P, 8], FP32, name="tB", tag="tB")
            nc.sync.dma_start(tA[:], scr_off[0:1024].rearrange("(u p) -> p u", p=P))
            nc.sync.dma_start(tB[:], scr_off[1024:2048].rearrange("(u p) -> p u", p=P))

            def pix_chain(refs, tanh_t, pref):
                def st(nm):
                    return s6.tile([P, 8], FP32, name=pref + nm, tag=pref + nm)
                pixv, w1, c0 = st("pix"), st("w1"), st("c0")
                c0c, c1c, v0, v1 = st("c0c"), st("c1c"), st("v0"), st("v1")
                tmp1, tmp2 = st("tmp1"), st("tmp2")
                nc.vector.tensor_tensor(pixv[:], refs[:], tanh_t[:], ALU.add)
                nc.vector.tensor_scalar(pixv[:], pixv[:], 504.0, 535.5,
                                        ALU.mult, ALU.add)
                # exact floor via the fp32 magic-constant round of (x - 0.5):
                # round(x-0.5) == floor(x) except at exact integers, where it
                # may give x-1 with frac 1.0 -- bilinear-equivalent.
                nc.vector.tensor_scalar(c0[:], pixv[:], -0.5, 12582912.0,
                                        ALU.add, ALU.add)
                nc.vector.tensor_scalar(c0[:], c0[:], -12582912.0, None, ALU.add)
                nc.vector.tensor_tensor(w1[:], pixv[:], c0[:], ALU.subtract)
                nc.vector.tensor_scalar(tmp1[:], c0[:], 0.0, None, ALU.is_ge)
                nc.vector.tensor_scalar(tmp2[:], c0[:], 63.0, None, ALU.is_le)
                nc.vector.tensor_tensor(v0[:], tmp1[:], tmp2[:], ALU.mult)
                nc.vector.tensor_scalar(tmp1[:], c0[:], -1.0, None, ALU.is_ge)
                nc.vector.tensor_scalar(tmp2[:], c0[:], 62.0, None, ALU.is_le)
                nc.vector.tensor_tensor(v1[:], tmp1[:], tmp2[:], ALU.mult)
                nc.vector.tensor_scalar(c0c[:], c0[:], 0.0, 63.0, ALU.max, ALU.min)
                nc.vector.tensor_scalar(c1c[:], c0c[:], 1.0, 63.0, ALU.add, ALU.min)
                return w1, c0c, c1c, v0, v1

            wy, y0c, y1c, vy0, vy1 = pix_chain(refsA, tA, "y")
            wx, x0c, x1c, vx0, vx1 = pix_chain(refsB, tB, "x")

            omx = s6.tile([P, 8], FP32, name="omx", tag="omx")
            omy = s6.tile([P, 8], FP32, name="omy", tag="omy")
            nc.vector.tensor_scalar(omx[:], wx[:], -1.0, 1.0, ALU.mult, ALU.add)
            nc.vector.tensor_scalar(omy[:], wy[:], -1.0, 1.0, ALU.mult, ALU.add)

            tmpx = s6.tile([P, 8], FP32, name="tmpx", tag="tmpx")
            idxs, wts = [], []
            for (cy, vy, wyy) in ((y0c, vy0, omy), (y1c, vy1, wy)):
                for (cx, vx, wxx) in ((x0c, vx0, omx), (x1c, vx1, wx)):
                    i = len(idxs)
                    idf = s6.tile([P, 8], FP32, name=f"idf{i}", tag=f"idf{i}")
                    idi = s6.tile([P, 8], I32, name=f"idi{i}", tag=f"idi{i}")
                    wt = s6.tile([P, 8], FP32, name=f"wt{i}", tag=f"wt{i}")
                    nc.vector.tensor_scalar(idf[:], cy[:], 32768.0, None, ALU.mult)
                    nc.vector.tensor_scalar(tmpx[:], cx[:], 512.0, None, ALU.mult)
                    nc.vector.tensor_tensor(idf[:], idf[:], tmpx[:], ALU.add)
                    nc.vector.tensor_copy(idi[:], idf[:])
                    nc.vector.tensor_tensor(wt[:], wxx[:], wyy[:], ALU.mult)
                    nc.vector.tensor_tensor(wt[:], wt[:], vx[:], ALU.mult)
                    nc.vector.tensor_tensor(wt[:], wt[:], vy[:], ALU.mult)
                    idxs.append(idi)
                    wts.append(wt)

            with tc.tile_pool(name="s7", bufs=8) as s7:
                # overlapping-window view of x: row i -> 1024 elements
                # [row i | row i+1]; corner pairs (x0,x0+1) share one gather.
                xflat = d_xgat[:].rearrange("r c -> (r c)").unsqueeze(-1)
                for u in range(8):
                    gs = []
                    for ci in (0, 2):   # idx of (y0,x0) and (y1,x0)
                        g = s7.tile([P, 2 * C], BF16, name=f"g{ci}", tag=f"g{ci}")
                        nc.gpsimd.indirect_dma_start(
                            out=g[:], out_offset=None, in_=xflat,
                            in_offset=bass.IndirectOffsetOnAxis(
                                ap=idxs[ci][:, u:u + 1], axis=0))
                        gs.append(g)
                    corners = [gs[0][:, 0:C], gs[0][:, C:2 * C],
                               gs[1][:, 0:C], gs[1][:, C:2 * C]]
                    acc = s7.tile([P, C], FP32, name="acc", tag="acc")
                    tmp = s7.tile([P, C], FP32, name="tmp", tag="tmp")
                    nc.vector.tensor_scalar(acc[:], corners[0],
                                            wts[0][:, u:u + 1], None, ALU.mult)
                    for ci in range(1, 3):
                        nc.vector.tensor_scalar(tmp[:], corners[ci],
                                                wts[ci][:, u:u + 1], None, ALU.mult)
                        nc.vector.tensor_tensor(acc[:], acc[:], tmp[:], ALU.add)
                    nc.vector.tensor_scalar(tmp[:], corners[3],
                                            wts[3][:, u:u + 1], None, ALU.mult)
                    nc.vector.tensor_tensor(interp[:, u * C:(u + 1) * C],
                                            acc[:], tmp[:], ALU.add)

        # ---------------- Stage 8: V^T, KH^T, VH65, QH^T --------------------
        with tc.tile_pool(name="s8p", bufs=8, space="PSUM") as s8p:
            for jp in range(4):
                for hh in range(2):
                    ps = s8p.tile([P, 512], FP32, name="vps", tag="s8ps")
                    for a in range(4):
                        nc.tensor.matmul(
                            ps[:],
                            w_sb["wv"][:, a * C + jp * P: a * C + (jp + 1) * P],
                            interp[:, (4 * hh + a) * C:(4 * hh + a + 1) * C],
                            start=(a == 0), stop=(a == 3))
                    nc.vector.tensor_copy(
                        vt[:, jp * KEYS + hh * 512: jp * KEYS + (hh + 1) * 512],
                        ps[:])
            for fp in range(4):
                for hh in range(2):
                    ps = s8p.tile([P, 512], FP32, name="kps", tag="s8ps")
                    for a in range(4):
                        nc.tensor.matmul(
                            ps[:],
                            w_sb["mk"][:, a * C + fp * P: a * C + (fp + 1) * P],
                            vt[:, a * KEYS + hh * 512: a * KEYS + (hh + 1) * 512],
                            start=(a == 0), stop=(a == 3))
                    nc.vector.tensor_copy(
                        kht[:, fp * KEYS + hh * 512: fp * KEYS + (hh + 1) * 512],
                        ps[:])
            vh_view = vh65[:].rearrange("p (kb n s65) -> p kb n s65", kb=8, n=8)
            nc.any.memset(vh_view[:, :, :, 64:65], 1.0)
            for kb in range(8):
                ps = s8p.tile([P, 512], FP32, name="vhps", tag="s8ps")
                for a in range(4):
                    nc.tensor.matmul(
                        ps[:],
                        vt[:, a * KEYS + kb * P: a * KEYS + (kb + 1) * P],
                        w_sb["mv"][:, a * C:(a + 1) * C],
                        start=(a == 0), stop=(a == 3))
                nc.vector.tensor_copy(
                    vh_view[:, kb, :, 0:64],
                    ps[:].rearrange("p (n d) -> p n d", n=8))
        pool_qt.release()

        # ---------------- Stage 9: attention --------------------------------
        pool_araw = tc.alloc_tile_pool(name="p_araw", bufs=1)
        araw = pool_araw.tile([P, 4 * TH], BF16, name="araw", tag="araw")
        with tc.tile_pool(name="s9e", bufs=2, space="PSUM") as s9e, \
             tc.tile_pool(name="s9a", bufs=4, space="PSUM") as s9a, \
             tc.tile_pool(name="s9", bufs=9) as s9, \
             tc.tile_pool(name="s9b", bufs=4) as s9b:
            for n in range(NH):
                ft, fr = n // 2, 64 * (n % 2)
                ptiles = []
                for kb in range(8):
                    pt = s9.tile([P, TH], BF16, name="pt", tag="pt")
                    for qh2 in range(2):
                        psl = s9e.tile([P, 1024], FP32, name="psl", tag="psl")
                        for ch2 in range(2):
                            ch = qh2 * 2 + ch2
                            nc.tensor.matmul(
                                psl[:, ch2 * 512:(ch2 + 1) * 512],
                                kht[fr:fr + 64, ft * KEYS + kb * P:
                                    ft * KEYS + (kb + 1) * P],
                                qht[fr:fr + 64, ft * TH + ch * 512:
                                    ft * TH + (ch + 1) * 512],
                                start=True, stop=True)
                        nc.scalar.activation(pt[:, qh2 * 1024:(qh2 + 1) * 1024],
                                             psl[:], ACTF.Exp)
                    ptiles.append(pt)
                for ch in range(4):
                    psa = s9a.tile([65, 512], FP32, name="psa", tag="psa")
                    for kb in range(8):
                        nc.tensor.matmul(
                            psa[:],
                            vh65[:, kb * 520 + 65 * n: kb * 520 + 65 * n + 65],
                            ptiles[kb][:, ch * 512:(ch + 1) * 512],
                            start=(kb == 0), stop=(kb == 7))
                    dcol = s9b.tile([1, 512], FP32, name="dcol", tag="dcol")
                    dsb = s9b.tile([64, 512], FP32, name="dsb", tag="dsb")
                    nc.vector.reciprocal(dcol[:], psa[64:65, :])
                    nc.gpsimd.partition_broadcast(dsb[:], dcol[:])
                    nc.vector.tensor_tensor(
                        araw[fr:fr + 64, ft * TH + ch * 512:
                             ft * TH + (ch + 1) * 512],
                        psa[0:64, :], dsb[:], ALU.mult)


        # ---------------- Stage 11: mo + residual + LN2 ---------------------
        pool_tail = tc.alloc_tile_pool(name="p_tail", bufs=1, side="right")
        z_buf = pool_tail.tile([P, 16 * C], FP32, name="z_buf", tag="z_buf")
        zlnt = pool_tail.tile([P, 4 * TH], BF16, name="zlnt", tag="zlnt")
        with tc.tile_pool(name="s11p", bufs=4, space="PSUM") as s11p, \
             tc.tile_pool(name="s11", bufs=4) as s11, \
             tc.tile_pool(name="s11s", bufs=4) as s11s:
            for tb in range(16):
                ps = s11p.tile([P, C], FP32, name="mops", tag="mops")
                for a in range(4):
                    nc.tensor.matmul(
                        ps[:],
                        araw[:, a * TH + tb * P: a * TH + (tb + 1) * P],
                        w_sb["mo"][:, a * C:(a + 1) * C],
                        start=(a == 0), stop=(a == 3))
                xrt = s11.tile([P, C], FP32, name="xrt", tag="xrt")
                nc.sync.dma_start(xrt[:], d_xres[tb * P:(tb + 1) * P, :])
                nc.vector.tensor_tensor(z_buf[:, tb * C:(tb + 1) * C],
                                        ps[:], xrt[:], ALU.add)
                stats = s11s.tile([P, 6], FP32, name="stats2", tag="stats2")
                aggr = s11s.tile([P, 2], FP32, name="aggr2", tag="aggr2")
                rstd = s11s.tile([P, 1], FP32, name="rstd2", tag="rstd2")
                eps = s11s.tile([P, 1], FP32, name="eps2", tag="eps2")
                zl = s11.tile([P, C], BF16, name="zl", tag="zl")
                nc.vector.bn_stats(stats[:], z_buf[:, tb * C:(tb + 1) * C])
                nc.vector.bn_aggr(aggr[:], stats[:])
                nc.any.memset(eps[:], 1e-5)
                nc.scalar.activation(rstd[:], aggr[:, 1:2], ACTF.Sqrt,
                                     bias=eps[:, 0:1], scale=1.0)
                nc.vector.reciprocal(rstd[:], rstd[:])
                nc.vector.tensor_scalar(zl[:], z_buf[:, tb * C:(tb + 1) * C],
                                        aggr[:, 0:1], rstd[:, 0:1],
                                        ALU.subtract, ALU.mult)
                zpst = s11p.tile([P, C], BF16, name="zpst", tag="zpst")
                for cp in range(4):
                    nc.tensor.transpose(zpst[:, cp * P:(cp + 1) * P],
                                        zl[:, cp * P:(cp + 1) * P], ident[:])
                nc.scalar.copy(
                    zlnt[:].rearrange("p (cp t) -> p cp t", cp=4)
                        [:, :, tb * P:(tb + 1) * P],
                    zpst[:].rearrange("p (cp t) -> p cp t", cp=4))
        pool_araw.release()

        # ---------------- Stage 12: MLP tail --------------------------------
        with tc.tile_pool(name="s12p", bufs=4, space="PSUM") as s12p, \
             tc.tile_pool(name="s12", bufs=4) as s12:
            for tb in range(16):
                ps = s12p.tile([P, C], FP32, name="fps", tag="fps")
                for a in range(4):
                    nc.tensor.matmul(
                        ps[:],
                        zlnt[:, a * TH + tb * P: a * TH + (tb + 1) * P],
                        w_sb["mlp"][:, a * C:(a + 1) * C],
                        start=(a == 0), stop=(a == 3))
                gl = s12.tile([P, C], FP32, name="gl", tag="gl")
                nc.scalar.activation(gl[:], ps[:], ACTF.Gelu)
                ot = s12.tile([P, C], FP32, name="ot", tag="ot")
                nc.vector.tensor_tensor(ot[:], gl[:],
                                        z_buf[:, tb * C:(tb + 1) * C], ALU.add)
                nc.sync.dma_start(d_out[tb * P:(tb + 1) * P, :], ot[:])
        pool_tail.release()
        pers.release()
        drs.release()

    nc.compile()
    return nc


# ---------------------------------------------------------------------------
# host side
# ---------------------------------------------------------------------------
_REF_VALS = (np.arange(32, dtype=np.float64) + 0.5) / 16.0 - 1.0


def make_in_maps(inputs):
    x = np.asarray(inputs["x"], dtype=np.float32)        # (4, 64, 64, 512)
    bf = ml_dtypes.bfloat16

    for nm in ("ln_b", "bq", "bv", "dw_b", "mq_b", "mk_b", "mv_b", "mo_b",
               "mlp_b"):
        assert np.all(np.asarray(inputs[nm]) == 0.0), f"nonzero bias {nm} unsupported"
    assert np.all(np.asarray(inputs["ln_g"]) == 1.0), "non-unit ln_g unsupported"

    w_bf = {
        "wq": np.asarray(inputs["wq"], np.float32).astype(bf),
        "wv": np.asarray(inputs["wv"], np.float32).astype(bf),
        "mq": np.asarray(inputs["mq_w"], np.float32).astype(bf),
        "mk": np.asarray(inputs["mk_w"], np.float32).astype(bf),
        "mv": np.asarray(inputs["mv_w"], np.float32).astype(bf),
        "mo": np.asarray(inputs["mo_w"], np.float32).astype(bf),
        "mlp": np.asarray(inputs["mlp_w"], np.float32).astype(bf),
    }
    dw = np.asarray(inputs["dw_w"], np.float32).reshape(C, 49)   # (512, 49)
    pw = np.asarray(inputs["pw_w"], np.float32)[:, :, 0, 0].T    # (512, 2)

    su = np.arange(1024)
    refsA = _REF_VALS[(su // 32)].astype(np.float32).reshape(8, P).T.copy()
    refsB = _REF_VALS[(su % 32)].astype(np.float32).reshape(8, P).T.copy()

    in_maps = []
    for core in range(8):
        b, h = core // 2, core % 2
        xb = x[b].reshape(T, C)
        x_rot = np.roll(xb, -256 * h, axis=1) if h else xb
        x_gat = np.concatenate([x_rot, np.zeros((1, C), np.float32)],
                               axis=0).astype(bf)
        dw_rot = np.roll(dw, -256 * h, axis=0) if h else dw
        pw_rot = np.roll(pw, -256 * h, axis=0) if h else pw
        dwdiag = np.zeros((49, 4, P, P), np.float32)
        ar = np.arange(P)
        for tap in range(49):
            for T4 in range(4):
                dwdiag[tap, T4, ar, ar] = dw_rot[T4 * P + ar, tap]
        # partition-major: [p, (tap*4+T4)*P + q]
        dwdiag_t = dwdiag.reshape(49 * 4, P, P).transpose(1, 0, 2) \
                         .reshape(P, 49 * 4 * P)
        m = {
            "x_rot": np.ascontiguousarray(x_rot),
            "x_gat": np.ascontiguousarray(x_gat),
            "x_res": np.ascontiguousarray(xb[TH * h: TH * (h + 1)]),
            "dwdiag": np.ascontiguousarray(dwdiag_t).astype(bf),
            "pw": np.ascontiguousarray(pw_rot).astype(bf),
            "refsA": refsA,
            "refsB": refsB,
        }
        m.update(w_bf)
        in_maps.append(m)
    return in_maps


_NC_CACHE = {}


def get_program():
    if "nc" not in _NC_CACHE:
        _NC_CACHE["nc"] = build_program()
    return _NC_CACHE["nc"]


def kernel(**inputs) -> np.ndarray:
    from concourse.bass_utils import run_bass_kernel_spmd

    nc = get_program()
    in_maps = make_in_maps(inputs)
    res = run_bass_kernel_spmd(nc, in_maps, core_ids=list(range(8)))
    out = np.zeros((4, T, C), np.float32)
    for core in range(8):
        b, h = core // 2, core % 2
        out[b, TH * h: TH * (h + 1)] = res.results[core]["out"]
    return out.reshape(4, 64, 64, C)



# revision 17
# speedup vs baseline: 1.0446x; 1.0446x over previous
"""Trainium2 Bass kernel for nn_Deformable_33397665693799.

Strategy (8 cores, B=4): 2 cores per batch; each core computes the full
per-batch shared pipeline (LN1 -> Q -> depthwise-conv offsets -> deformed
grid-sample gather -> V/KH/VH) and half of the 4096 attention queries
(attention + output projections + MLP tail).  The query halves are selected
without any per-core program differences by feeding each core a
channel-ROTATED copy of x: the faithful torch-style reshape scramble maps
token t = 8c+s to LN-channel c, so rotating x's channels by 256h makes each
core's "first 2048 tokens" equal true tokens [2048h, 2048h+2048).  The
depthwise-conv weights / 1x1-offset-conv weights are rotated identically on
the host, and grid-sample keys are an (order-irrelevant) permutation, so the
single SPMD program is exact for both halves.

All cores run one identical Bass/Tile program; only input data differs.
"""
import sys
import os

sys.path.insert(0, "/opt/trn_rl_repo")

import numpy as np
import ml_dtypes

import concourse.bass as bass
import concourse.mybir as mybir
import concourse.tile as tile
from concourse import bacc
from concourse.masks import make_identity

FP32 = mybir.dt.float32
BF16 = mybir.dt.bfloat16
I32 = mybir.dt.int32
ALU = mybir.AluOpType
ACTF = mybir.ActivationFunctionType

P = 128
C = 512            # channels
T = 4096           # tokens per batch
TH = 2048          # tokens per core (query half)
KEYS = 1024        # attention keys
NH = 8             # heads
PADW = 70          # padded conv row width  (x in [-3, 67))
PADH = 70          # padded conv rows       (y in [-3, 67))
CONVF = PADH * PADW


def build_program():
    nc = bacc.Bacc("TRN2", target_bir_lowering=False, debug=False)

    # ---------------- DRAM I/O ----------------
    d_x = nc.dram_tensor("x_rot", [T, C], FP32, kind="ExternalInput")
    d_xgat = nc.dram_tensor("x_gat", [T + 1, C], BF16, kind="ExternalInput")
    d_xres = nc.dram_tensor("x_res", [TH, C], FP32, kind="ExternalInput")
    d_w = {}
    for name in ("wq", "wv", "mq", "mk", "mv", "mo", "mlp"):
        d_w[name] = nc.dram_tensor(name, [C, C], BF16, kind="ExternalInput")
    d_dwdiag = nc.dram_tensor("dwdiag", [P, 49 * 4 * P], BF16, kind="ExternalInput")
    d_pw = nc.dram_tensor("pw", [C, 2], BF16, kind="ExternalInput")
    d_refsA = nc.dram_tensor("refsA", [P, 8], FP32, kind="ExternalInput")
    d_refsB = nc.dram_tensor("refsB", [P, 8], FP32, kind="ExternalInput")
    d_out = nc.dram_tensor("out", [TH, C], FP32, kind="ExternalOutput")

    with tile.TileContext(nc) as tc:
        drs = tc.alloc_tile_pool(name="drs", bufs=1, space="DRAM")
        pers = tc.alloc_tile_pool(name="persist", bufs=1)

        q_scr4 = [drs.tile([T // 4, C], BF16, name=f"q_scr{i}")
                  for i in range(4)]
        scr_off = drs.tile([2048], FP32, name="scr_off")
        scr_dens = [drs.tile([8 * 512], FP32, name=f"scr_dens{i}")
                    for i in range(4)]
        scr_rdens = [drs.tile([8 * 512], FP32, name=f"scr_rdens{i}")
                     for i in range(4)]

        # ---- persistent (whole-kernel) SBUF ----
        w_sb = {}
        for name in ("wq", "wv", "mq", "mk", "mv", "mo", "mlp"):
            w_sb[name] = pers.tile([P, 4 * C], BF16, name=f"w_{name}",
                                   tag=f"w_{name}")
            for a in range(4):
                nc.sync.dma_start(w_sb[name][:, a * C:(a + 1) * C],
                                  d_w[name][a * P:(a + 1) * P, :])
        pw_sb = pers.tile([P, 8], BF16, name="pw_sb", tag="pw_sb")
        for a in range(4):
            nc.sync.dma_start(pw_sb[:, a * 2:(a + 1) * 2],
                              d_pw[a * P:(a + 1) * P, :])
        refsA = pers.tile([P, 8], FP32, name="refsA", tag="refsA")
        refsB = pers.tile([P, 8], FP32, name="refsB", tag="refsB")
        nc.sync.dma_start(refsA[:], d_refsA[:])
        nc.sync.dma_start(refsB[:], d_refsB[:])
        ident = pers.tile([P, P], BF16, name="ident", tag="ident")
        make_identity(nc, ident[:])

        kht = pers.tile([P, 4 * KEYS], BF16, name="kht", tag="kht")
        vt = pers.tile([P, 4 * KEYS], BF16, name="vt", tag="vt")
        vh65 = pers.tile([P, 8 * 520], BF16, name="vh65", tag="vh65")
        interp = pers.tile([P, 8 * C], BF16, name="interp", tag="interp")
        qht = pers.tile([P, 4 * TH], BF16, name="qht", tag="qht")

        # ---- scoped big buffers ----
        pool_ln1 = tc.alloc_tile_pool(name="p_ln1", bufs=1)
        ln1 = pool_ln1.tile([P, 32 * C], BF16, name="ln1", tag="ln1")
        pool_qt = tc.alloc_tile_pool(name="p_qt", bufs=1, side="right")
        qt = pool_qt.tile([P, 4 * T], BF16, name="qt", tag="qt")

        # ---------------- Stage 1: LN1 ----------------
        with tc.tile_pool(name="s1", bufs=3) as s1, \
             tc.tile_pool(name="s1s", bufs=4) as s1s:
            for pi in range(32):
                xt = s1.tile([P, C], FP32, name="xt", tag="xt")
                nc.sync.dma_start(xt[:], d_x[pi * P:(pi + 1) * P, :])
                stats = s1s.tile([P, 6], FP32, name="stats", tag="stats")
                aggr = s1s.tile([P, 2], FP32, name="aggr", tag="aggr")
                rstd = s1s.tile([P, 1], FP32, name="rstd", tag="rstd")
                eps = s1s.tile([P, 1], FP32, name="eps", tag="eps")
                nb = s1s.tile([P, 1], FP32, name="nb", tag="nb")
                nc.vector.bn_stats(stats[:], xt[:])
                nc.vector.bn_aggr(aggr[:], stats[:])
                nc.any.memset(eps[:], 1e-5)
                nc.scalar.activation(rstd[:], aggr[:, 1:2], ACTF.Sqrt,
                                     bias=eps[:, 0:1], scale=1.0)
                nc.vector.reciprocal(rstd[:], rstd[:])
                nc.vector.tensor_scalar(nb[:], aggr[:, 0:1], -1.0, None,
                                        ALU.mult)
                nc.vector.tensor_tensor(nb[:], nb[:], rstd[:], ALU.mult)
                nc.scalar.activation(ln1[:, pi * C:(pi + 1) * C], xt[:],
                                     ACTF.Identity, bias=nb[:, 0:1],
                                     scale=rstd[:, 0:1])

        # ---------------- Stage 2: Q^T ----------------
        qtv = qt[:].rearrange("p (cp c s8) -> p cp c s8", cp=4, s8=8)
        with tc.tile_pool(name="s2p", bufs=8, space="PSUM") as s2p:
            for cp in range(4):
                psums = [s2p.tile([P, C], FP32, name="qpsum", tag="qpsum")
                         for _ in range(8)]
                for a in range(4):
                    lhsT = w_sb["wq"][:, a * C + cp * P: a * C + (cp + 1) * P]
                    for s in range(8):
                        nc.tensor.matmul(
                            psums[s][:], lhsT,
                            ln1[:, (4 * s + a) * C:(4 * s + a + 1) * C],
                            start=(a == 0), stop=(a == 3))
                for s in range(8):
                    nc.vector.tensor_copy(qtv[:, cp, :, s], psums[s][:])
        pool_ln1.release()

        # ---------------- Stage 3: Q^T -> q_scr -> padded conv input --------
        pool_conv = tc.alloc_tile_pool(name="p_conv", bufs=1, side="right")
        conv_in4 = [pool_conv.tile([P, CONVF], BF16, name=f"conv_in{i}",
                                   tag=f"conv_in{i}") for i in range(4)]
        gelu4 = [pool_conv.tile([P, 1024], BF16, name=f"gelu{i}",
                                tag=f"gelu{i}") for i in range(4)]
        for i in range(4):
            nc.gpsimd.memset(conv_in4[i][:], 0.0)
        with tc.tile_pool(name="s3", bufs=4) as s3, \
             tc.tile_pool(name="s3p", bufs=4, space="PSUM") as s3p:
            for tt in range(32):
                pst = s3p.tile([P, C], BF16, name="pst", tag="pst")
                for cp in range(4):
                    nc.tensor.transpose(
                        pst[:, cp * P:(cp + 1) * P],
                        qt[:, cp * T + tt * P: cp * T + (tt + 1) * P],
                        ident[:])
                qtmp = s3.tile([P, C], BF16, name="qtmp", tag="qtmp")
                nc.scalar.copy(qtmp[:], pst[:])
                T4, rr = tt // 8, tt % 8
                nc.sync.dma_start(q_scr4[T4][rr * P:(rr + 1) * P, :], qtmp[:])
                if rr == 7:
                    civ = conv_in4[T4][:] \
                        .rearrange("p (y x) -> p y x", y=PADH)
                    qs_view = q_scr4[T4][:] \
                        .rearrange("(c s) j -> c (s j)", s=8) \
                        .rearrange("c (y x) -> c y x", y=64)
                    nc.sync.dma_start(civ[:, 3:67, 3:67], qs_view[:])

        # ---------------- Stage 4: depthwise conv 7x7 stride 2 -------------
        with tc.tile_pool(name="s4d", bufs=6) as s4d, \
             tc.tile_pool(name="s4p", bufs=4, space="PSUM") as s4p:
            cpsum = [s4p.tile([P, 1024], FP32, name="cpsum", tag="cpsum")
                     for _ in range(4)]
            dg8 = None
            for tap in range(49):
                ky, kx = tap // 7, tap % 7
                for T4 in range(4):
                    k8 = (tap * 4 + T4) % 8
                    if k8 == 0:
                        dg8 = s4d.tile([P, 8 * P], BF16, name="dg8", tag="dg8")
                        col0 = (tap * 4 + T4) * P
                        ncol = min(8 * P, 49 * 4 * P - col0)
                        nc.sync.dma_start(dg8[:, 0:ncol],
                                          d_dwdiag[:, col0:col0 + ncol])
                    dg = dg8[:, k8 * P:(k8 + 1) * P]
                    civ = conv_in4[T4][:] \
                        .rearrange("p (y x) -> p y x", y=PADH)
                    rv = civ[:, ky:ky + 64:2, kx:kx + 64:2]
                    nc.tensor.matmul(cpsum[T4][:, 0:512], dg,
                                     rv[:, 0:16, :],
                                     start=(tap == 0), stop=(tap == 48))
                    nc.tensor.matmul(cpsum[T4][:, 512:1024], dg,
                                     rv[:, 16:32, :],
                                     start=(tap == 0), stop=(tap == 48))
            for T4 in range(4):
                nc.scalar.activation(gelu4[T4][:], cpsum[T4][:], ACTF.Gelu)

        # ---------------- Stage 5: offsets + tanh ---------------------------
        with tc.tile_pool(name="s5p", bufs=1, space="PSUM") as s5p, \
             tc.tile_pool(name="s5", bufs=1) as s5:
            opsum = s5p.tile([2, 1024], FP32, name="opsum", tag="opsum")
            for T4 in range(4):
                for half in range(2):
                    nc.tensor.matmul(
                        opsum[:, half * 512:(half + 1) * 512],
                        pw_sb[:, T4 * 2:(T4 + 1) * 2],
                        gelu4[T4][:, half * 512:(half + 1) * 512],
                        start=(T4 == 0), stop=(T4 == 3))
            off_t = s5.tile([2, 1024], FP32, name="off_t", tag="off_t")
            nc.scalar.activation(off_t[:], opsum[:], ACTF.Tanh)
            nc.sync.dma_start(scr_off[0:1024], off_t[0:1, :])
            nc.sync.dma_start(scr_off[1024:2048], off_t[1:2, :])
        pool_conv.release()

        # ---------------- Stage 5b: QH^T (overlaps gather) ------------------
        with tc.tile_pool(name="s5q", bufs=4, space="PSUM") as s5q:
            for fp in range(4):
                for ch in range(4):
                    ps = s5q.tile([P, 512], FP32, name="qhps", tag="qhps")
                    for a in range(4):
                        nc.tensor.matmul(
                            ps[:],
                            w_sb["mq"][:, a * C + fp * P: a * C + (fp + 1) * P],
                            qt[:, a * T + ch * 512:a * T + (ch + 1) * 512],
                            start=(a == 0), stop=(a == 3))
                    nc.vector.tensor_copy(
                        qht[:, fp * TH + ch * 512: fp * TH + (ch + 1) * 512],
                        ps[:])

        # ---------------- Stage 6+7: pixel math, gather, bilinear -----------
        with tc.tile_pool(name="s6", bufs=1) as s6:
            tA = s6.tile([P, 8], FP32, name="tA", tag="tA")
            tB = s6.tile([P, 8], FP32, name="tB", tag="tB")
            nc.sync.dma_start(tA[:], scr_off[0:1024].rearrange("(u p) -> p u", p=P))
            nc.sync.dma_start(tB[:], scr_off[1024:2048].rearrange("(u p) -> p u", p=P))

            def pix_chain(refs, tanh_t, pref):
                def st(nm):
                    return s6.tile([P, 8], FP32, name=pref + nm, tag=pref + nm)
                pixv, w1, c0 = st("pix"), st("w1"), st("c0")
                c0c, c1c, v0, v1 = st("c0c"), st("c1c"), st("v0"), st("v1")
                tmp1, tmp2 = st("tmp1"), st("tmp2")
                nc.vector.tensor_tensor(pixv[:], refs[:], tanh_t[:], ALU.add)
                nc.vector.tensor_scalar(pixv[:], pixv[:], 504.0, 535.5,
                                        ALU.mult, ALU.add)
                # exact floor via the fp32 magic-constant round of (x - 0.5):
                # round(x-0.5) == floor(x) except at exact integers, where it
                # may give x-1 with frac 1.0 -- bilinear-equivalent.
                nc.vector.tensor_scalar(c0[:], pixv[:], -0.5, 12582912.0,
                                        ALU.add, ALU.add)
                nc.vector.tensor_scalar(c0[:], c0[:], -12582912.0, None, ALU.add)
                nc.vector.tensor_tensor(w1[:], pixv[:], c0[:], ALU.subtract)
                nc.vector.tensor_scalar(tmp1[:], c0[:], 0.0, None, ALU.is_ge)
                nc.vector.tensor_scalar(tmp2[:], c0[:], 63.0, None, ALU.is_le)
                nc.vector.tensor_tensor(v0[:], tmp1[:], tmp2[:], ALU.mult)
                nc.vector.tensor_scalar(tmp1[:], c0[:], -1.0, None, ALU.is_ge)
                nc.vector.tensor_scalar(tmp2[:], c0[:], 62.0, None, ALU.is_le)
                nc.vector.tensor_tensor(v1[:], tmp1[:], tmp2[:], ALU.mult)
                nc.vector.tensor_scalar(c0c[:], c0[:], 0.0, 63.0, ALU.max, ALU.min)
                nc.vector.tensor_scalar(c1c[:], c0c[:], 1.0, 63.0, ALU.add, ALU.min)
                return w1, c0c, c1c, v0, v1

            wy, y0c, y1c, vy0, vy1 = pix_chain(refsA, tA, "y")
            wx, x0c, x1c, vx0, vx1 = pix_chain(refsB, tB, "x")

            omx = s6.tile([P, 8], FP32, name="omx", tag="omx")
            omy = s6.tile([P, 8], FP32, name="omy", tag="omy")
            nc.vector.tensor_scalar(omx[:], wx[:], -1.0, 1.0, ALU.mult, ALU.add)
            nc.vector.tensor_scalar(omy[:], wy[:], -1.0, 1.0, ALU.mult, ALU.add)

            tmpx = s6.tile([P, 8], FP32, name="tmpx", tag="tmpx")
            idxs, wts = [], []
            for (cy, vy, wyy) in ((y0c, vy0, omy), (y1c, vy1, wy)):
                for (cx, vx, wxx) in ((x0c, vx0, omx), (x1c, vx1, wx)):
                    i = len(idxs)
                    idf = s6.tile([P, 8], FP32, name=f"idf{i}", tag=f"idf{i}")
                    idi = s6.tile([P, 8], I32, name=f"idi{i}", tag=f"idi{i}")
                    wt = s6.tile([P, 8], FP32, name=f"wt{i}", tag=f"wt{i}")
                    nc.vector.tensor_scalar(idf[:], cy[:], 32768.0, None, ALU.mult)
                    nc.vector.tensor_scalar(tmpx[:], cx[:], 512.0, None, ALU.mult)
                    nc.vector.tensor_tensor(idf[:], idf[:], tmpx[:], ALU.add)
                    nc.vector.tensor_copy(idi[:], idf[:])
                    nc.vector.tensor_tensor(wt[:], wxx[:], wyy[:], ALU.mult)
                    nc.vector.tensor_tensor(wt[:], wt[:], vx[:], ALU.mult)
                    nc.vector.tensor_tensor(wt[:], wt[:], vy[:], ALU.mult)
                    idxs.append(idi)
                    wts.append(wt)

            with tc.tile_pool(name="s7", bufs=8) as s7:
                # overlapping-window view of x: row i -> 1024 elements
                # [row i | row i+1]; corner pairs (x0,x0+1) share one gather.
                xflat = d_xgat[:].rearrange("r c -> (r c)").unsqueeze(-1)
                for u in range(8):
                    gs = []
                    for ci in (0, 2):   # idx of (y0,x0) and (y1,x0)
                        g = s7.tile([P, 2 * C], BF16, name=f"g{ci}", tag=f"g{ci}")
                        nc.gpsimd.indirect_dma_start(
                            out=g[:], out_offset=None, in_=xflat,
                            in_offset=bass.IndirectOffsetOnAxis(
                                ap=idxs[ci][:, u:u + 1], axis=0))
                        gs.append(g)
                    corners = [gs[0][:, 0:C], gs[0][:, C:2 * C],
                               gs[1][:, 0:C], gs[1][:, C:2 * C]]
                    acc = s7.tile([P, C], FP32, name="acc", tag="acc")
                    tmp = s7.tile([P, C], FP32, name="tmp", tag="tmp")
                    nc.vector.tensor_scalar(acc[:], corners[0],
                                            wts[0][:, u:u + 1], None, ALU.mult)
                    for ci in range(1, 3):
                        nc.vector.tensor_scalar(tmp[:], corners[ci],
                                                wts[ci][:, u:u + 1], None, ALU.mult)
                        nc.vector.tensor_tensor(acc[:], acc[:], tmp[:], ALU.add)
                    nc.vector.tensor_scalar(tmp[:], corners[3],
                                            wts[3][:, u:u + 1], None, ALU.mult)
                    nc.vector.tensor_tensor(interp[:, u * C:(u + 1) * C],
                                            acc[:], tmp[:], ALU.add)

        # ---------------- Stage 8: V^T, KH^T, VH65, QH^T --------------------
        with tc.tile_pool(name="s8p", bufs=8, space="PSUM") as s8p:
            for jp in range(4):
                for hh in range(2):
                    ps = s8p.tile([P, 512], FP32, name="vps", tag="s8ps")
                    for a in range(4):
                        nc.tensor.matmul(
                            ps[:],
                            w_sb["wv"][:, a * C + jp * P: a * C + (jp + 1) * P],
                            interp[:, (4 * hh + a) * C:(4 * hh + a + 1) * C],
                            start=(a == 0), stop=(a == 3))
                    nc.vector.tensor_copy(
                        vt[:, jp * KEYS + hh * 512: jp * KEYS + (hh + 1) * 512],
                        ps[:])
            for fp in range(4):
                for hh in range(2):
                    ps = s8p.tile([P, 512], FP32, name="kps", tag="s8ps")
                    for a in range(4):
                        nc.tensor.matmul(
                            ps[:],
                            w_sb["mk"][:, a * C + fp * P: a * C + (fp + 1) * P],
                            vt[:, a * KEYS + hh * 512: a * KEYS + (hh + 1) * 512],
                            start=(a == 0), stop=(a == 3))
                    nc.vector.tensor_copy(
                        kht[:, fp * KEYS + hh * 512: fp * KEYS + (hh + 1) * 512],
                        ps[:])
            vh_view = vh65[:].rearrange("p (kb n s65) -> p kb n s65", kb=8, n=8)
            nc.any.memset(vh_view[:, :, :, 64:65], 1.0)
            for kb in range(8):
                ps = s8p.tile([P, 512], FP32, name="vhps", tag="s8ps")
                for a in range(4):
                    nc.tensor.matmul(
                        ps[:],
                        vt[:, a * KEYS + kb * P: a * KEYS + (kb + 1) * P],
                        w_sb["mv"][:, a * C:(a + 1) * C],
                        start=(a == 0), stop=(a == 3))
                nc.vector.tensor_copy(
                    vh_view[:, kb, :, 0:64],
                    ps[:].rearrange("p (n d) -> p n d", n=8))
        pool_qt.release()

        # ---------------- Stage 9: attention --------------------------------
        pool_araw = tc.alloc_tile_pool(name="p_araw", bufs=1)
        araw = pool_araw.tile([P, 4 * TH], BF16, name="araw", tag="araw")
        with tc.tile_pool(name="s9e", bufs=2, space="PSUM") as s9e, \
             tc.tile_pool(name="s9a", bufs=4, space="PSUM") as s9a, \
             tc.tile_pool(name="s9", bufs=9) as s9, \
             tc.tile_pool(name="s9b", bufs=4) as s9b, \
             tc.tile_pool(name="s9r", bufs=8) as s9r, \
             tc.tile_pool(name="s9c", bufs=2) as s9c:
            for n in range(NH):
                ft, fr = n // 2, 64 * (n % 2)
                ptiles = []
                for kb in range(8):
                    pt = s9.tile([P, TH], BF16, name="pt", tag="pt")
                    for qh2 in range(2):
                        psl = s9e.tile([P, 1024], FP32, name="psl", tag="psl")
                        for ch2 in range(2):
                            ch = qh2 * 2 + ch2
                            nc.tensor.matmul(
                                psl[:, ch2 * 512:(ch2 + 1) * 512],
                                kht[fr:fr + 64, ft * KEYS + kb * P:
                                    ft * KEYS + (kb + 1) * P],
                                qht[fr:fr + 64, ft * TH + ch * 512:
                                    ft * TH + (ch + 1) * 512],
                                start=True, stop=True)
                        nc.scalar.activation(pt[:, qh2 * 1024:(qh2 + 1) * 1024],
                                             psl[:], ACTF.Exp)
                    ptiles.append(pt)
                for ch in range(4):
                    psa = s9a.tile([65, 512], FP32, name="psa", tag="psa")
                    for kb in range(8):
                        nc.tensor.matmul(
                            psa[:],
                            vh65[:, kb * 520 + 65 * n: kb * 520 + 65 * n + 65],
                            ptiles[kb][:, ch * 512:(ch + 1) * 512],
                            start=(kb == 0), stop=(kb == 7))
                    dcol = s9b.tile([1, 512], FP32, name="dcol", tag="dcol")
                    nc.vector.tensor_copy(dcol[:], psa[64:65, :])
                    nc.sync.dma_start(
                        scr_dens[ft][((n % 2) * 4 + ch) * 512:
                                     ((n % 2) * 4 + ch + 1) * 512],
                        dcol[0:1, :])
                    nc.vector.tensor_copy(
                        araw[fr:fr + 64, ft * TH + ch * 512:
                             ft * TH + (ch + 1) * 512],
                        psa[0:64, :])
                if n % 2 == 1:
                    dload = s9c.tile([P, 32], FP32, name="dload", tag="dload")
                    dview = scr_dens[ft].rearrange("(r cb p) -> p (r cb)",
                                                   r=8, p=P)
                    rdview = scr_rdens[ft].rearrange("(r cb p) -> p (r cb)",
                                                     r=8, p=P)
                    nc.sync.dma_start(dload[:], dview)
                    nc.vector.reciprocal(dload[:], dload[:])
                    nc.sync.dma_start(rdview, dload[:])
                    dsbf = s9c.tile([P, TH], FP32, name="dsbf", tag="dsbf")
                    for half in range(2):
                        for ch in range(4):
                            rrow = s9r.tile([1, 512], FP32, name="rrow",
                                            tag="rrow")
                            nc.sync.dma_start(
                                rrow[0:1, :],
                                scr_rdens[ft][(half * 4 + ch) * 512:
                                              (half * 4 + ch + 1) * 512])
                            nc.gpsimd.partition_broadcast(
                                dsbf[64 * half:64 * half + 64,
                                     ch * 512:(ch + 1) * 512], rrow[:])
                    nc.vector.tensor_tensor(
                        araw[:, ft * TH:(ft + 1) * TH],
                        araw[:, ft * TH:(ft + 1) * TH], dsbf[:], ALU.mult)


        # ---------------- Stage 11: mo + residual + LN2 ---------------------
        pool_tail = tc.alloc_tile_pool(name="p_tail", bufs=1, side="right")
        z_buf = pool_tail.tile([P, 16 * C], FP32, name="z_buf", tag="z_buf")
        zlnt = pool_tail.tile([P, 4 * TH], BF16, name="zlnt", tag="zlnt")
        with tc.tile_pool(name="s11p", bufs=4, space="PSUM") as s11p, \
             tc.tile_pool(name="s11", bufs=4) as s11, \
             tc.tile_pool(name="s11s", bufs=4) as s11s:
            for tb in range(16):
                ps = s11p.tile([P, C], FP32, name="mops", tag="mops")
                for a in range(4):
                    nc.tensor.matmul(
                        ps[:],
                        araw[:, a * TH + tb * P: a * TH + (tb + 1) * P],
                        w_sb["mo"][:, a * C:(a + 1) * C],
                        start=(a == 0), stop=(a == 3))
                xrt = s11.tile([P, C], FP32, name="xrt", tag="xrt")
                nc.sync.dma_start(xrt[:], d_xres[tb * P:(tb + 1) * P, :])
                nc.vector.tensor_tensor(z_buf[:, tb * C:(tb + 1) * C],
                                        ps[:], xrt[:], ALU.add)
                stats = s11s.tile([P, 6], FP32, name="stats2", tag="stats2")
                aggr = s11s.tile([P, 2], FP32, name="aggr2", tag="aggr2")
                rstd = s11s.tile([P, 1], FP32, name="rstd2", tag="rstd2")
                eps = s11s.tile([P, 1], FP32, name="eps2", tag="eps2")
                zl = s11.tile([P, C], BF16, name="zl", tag="zl")
                nc.vector.bn_stats(stats[:], z_buf[:, tb * C:(tb + 1) * C])
                nc.vector.bn_aggr(aggr[:], stats[:])
                nc.any.memset(eps[:], 1e-5)
                nc.scalar.activation(rstd[:], aggr[:, 1:2], ACTF.Sqrt,
                                     bias=eps[:, 0:1], scale=1.0)
                nc.vector.reciprocal(rstd[:], rstd[:])
                nc.vector.tensor_scalar(zl[:], z_buf[:, tb * C:(tb + 1) * C],
                                        aggr[:, 0:1], rstd[:, 0:1],
                                        ALU.subtract, ALU.mult)
                zpst = s11p.tile([P, C], BF16, name="zpst", tag="zpst")
                for cp in range(4):
                    nc.tensor.transpose(zpst[:, cp * P:(cp + 1) * P],
                                        zl[:, cp * P:(cp + 1) * P], ident[:])
                nc.scalar.copy(
                    zlnt[:].rearrange("p (cp t) -> p cp t", cp=4)
                        [:, :, tb * P:(tb + 1) * P],
                    zpst[:].rearrange("p (cp t) -> p cp t", cp=4))
        pool_araw.release()

        # ---------------- Stage 12: MLP tail --------------------------------
        with tc.tile_pool(name="s12p", bufs=4, space="PSUM") as s12p, \
             tc.tile_pool(name="s12", bufs=4) as s12:
            for tb in range(16):
                ps = s12p.tile([P, C], FP32, name="fps", tag="fps")
                for a in range(4):
                    nc.tensor.matmul(
                        ps[:],
                        zlnt[:, a * TH + tb * P: a * TH + (tb + 1) * P],
                        w_sb["mlp"][:, a * C:(a + 1) * C],
                        start=(a == 0), stop=(a == 3))
                gl = s12.tile([P, C], FP32, name="gl", tag="gl")
                nc.scalar.activation(gl[:], ps[:], ACTF.Gelu)
                ot = s12.tile([P, C], FP32, name="ot", tag="ot")
                nc.vector.tensor_tensor(ot[:], gl[:],
                                        z_buf[:, tb * C:(tb + 1) * C], ALU.add)
                nc.sync.dma_start(d_out[tb * P:(tb + 1) * P, :], ot[:])
        pool_tail.release()
        pers.release()
        drs.release()

    nc.compile()
    return nc


# ---------------------------------------------------------------------------
# host side
# ---------------------------------------------------------------------------
_REF_VALS = (np.arange(32, dtype=np.float64) + 0.5) / 16.0 - 1.0


def make_in_maps(inputs):
    x = np.asarray(inputs["x"], dtype=np.float32)        # (4, 64, 64, 512)
    bf = ml_dtypes.bfloat16

    for nm in ("ln_b", "bq", "bv", "dw_b", "mq_b", "mk_b", "mv_b", "mo_b",
               "mlp_b"):
        assert np.all(np.asarray(inputs[nm]) == 0.0), f"nonzero bias {nm} unsupported"
    assert np.all(np.asarray(inputs["ln_g"]) == 1.0), "non-unit ln_g unsupported"

    w_bf = {
        "wq": np.asarray(inputs["wq"], np.float32).astype(bf),
        "wv": np.asarray(inputs["wv"], np.float32).astype(bf),
        "mq": np.asarray(inputs["mq_w"], np.float32).astype(bf),
        "mk": np.asarray(inputs["mk_w"], np.float32).astype(bf),
        "mv": np.asarray(inputs["mv_w"], np.float32).astype(bf),
        "mo": np.asarray(inputs["mo_w"], np.float32).astype(bf),
        "mlp": np.asarray(inputs["mlp_w"], np.float32).astype(bf),
    }
    dw = np.asarray(inputs["dw_w"], np.float32).reshape(C, 49)   # (512, 49)
    pw = np.asarray(inputs["pw_w"], np.float32)[:, :, 0, 0].T    # (512, 2)

    su = np.arange(1024)
    refsA = _REF_VALS[(su // 32)].astype(np.float32).reshape(8, P).T.copy()
    refsB = _REF_VALS[(su % 32)].astype(np.float32).reshape(8, P).T.copy()

    in_maps = []
    for core in range(8):
        b, h = core // 2, core % 2
        xb = x[b].reshape(T, C)
        x_rot = np.roll(xb, -256 * h, axis=1) if h else xb
        x_gat = np.concatenate([x_rot, np.zeros((1, C), np.float32)],
                               axis=0).astype(bf)
        dw_rot = np.roll(dw, -256 * h, axis=0) if h else dw
        pw_rot = np.roll(pw, -256 * h, axis=0) if h else pw
        dwdiag = np.zeros((49, 4, P, P), np.float32)
        ar = np.arange(P)
        for tap in range(49):
            for T4 in range(4):
                dwdiag[tap, T4, ar, ar] = dw_rot[T4 * P + ar, tap]
        # partition-major: [p, (tap*4+T4)*P + q]
        dwdiag_t = dwdiag.reshape(49 * 4, P, P).transpose(1, 0, 2) \
                         .reshape(P, 49 * 4 * P)
        m = {
            "x_rot": np.ascontiguousarray(x_rot),
            "x_gat": np.ascontiguousarray(x_gat),
            "x_res": np.ascontiguousarray(xb[TH * h: TH * (h + 1)]),
            "dwdiag": np.ascontiguousarray(dwdiag_t).astype(bf),
            "pw": np.ascontiguousarray(pw_rot).astype(bf),
            "refsA": refsA,
            "refsB": refsB,
        }
        m.update(w_bf)
        in_maps.append(m)
    return in_maps


_NC_CACHE = {}


def get_program():
    if "nc" not in _NC_CACHE:
        _NC_CACHE["nc"] = build_program()
    return _NC_CACHE["nc"]


def kernel(**inputs) -> np.ndarray:
    from concourse.bass_utils import run_bass_kernel_spmd

    nc = get_program()
    in_maps = make_in_maps(inputs)
    res = run_bass_kernel_spmd(nc, in_maps, core_ids=list(range(8)))
    out = np.zeros((4, T, C), np.float32)
    for core in range(8):
        b, h = core // 2, core % 2
        out[b, TH * h: TH * (h + 1)] = res.results[core]["out"]
    return out.reshape(4, 64, 64, C)

